# revision 2
# baseline (speedup 1.0000x reference)
"""Trainium2 Bass kernel for nn_GNN_37615323579234 (gnn_message_passing).

Math (reference, N=8192, D=64, 4 layers; layer-3 A@H products are dead code):
    l=0..3:  H_cl = relu(X1@w1+b1) + relu(X2@w2+b2);  H_ue = relu(Xue@w3+b3)
             X1 = A_cl@H_cl;  X2 = A_ue@H_ue;  Xue = A_ue@H_cl
    out = relu(colsum(H_cl3) @ Qw1 + Qb1) @ Qw2 + Qb2      # [1,1]

Sharding: row-shard A_cl/A_ue over 8 cores (1024 rows each).  Host feeds each
core its A row-block TRANSPOSED and cast to bf16 ([8192,1024] contiguous) so
the contraction dim lands on SBUF partitions with line-rate DMA and half the
HBM traffic.  Big matmuls compute the output TRANSPOSED: stationary = H k-tile
(natural layout, bf16), moving = A^T k-tile (bf16), f32 PSUM.  A^T tiles are
DMA'd 4 k-tiles (1 MiB) per transfer.  H_ue|H_cl interleave per k-tile in one
SBUF buffer so the fused A_ue pass uses a single [128,128] stationary.  Biases
fold into the small matmuls via an appended ones-row.  Between layers: an
AllGather of the updated H blocks (DRAM bounce), AllReduce for the pooled vec.
"""

import os
import sys

for _p in ("/opt/trn_rl_repo", "/root/.axon_site/_ro/trn_rl_repo"):
    if os.path.isdir(_p) and _p not in sys.path:
        sys.path.insert(0, _p)

import numpy as np

N = 8192
D = 64
M = 8          # cores
R = N // M     # 1024 rows per core
P = 128        # partitions
KT = N // P    # 64 k-tiles
JT = R // P    # 8 row-tiles per core
KB = 4         # k-tiles per A-stream DMA (1 MiB in bf16)

LAST_EXEC_NS = None
LAST_PROFILE = None

_CACHED = None  # compile once per process


def _build_module():
    import concourse.bacc as bacc
    import concourse.mybir as mybir
    from concourse import tile

    f32 = mybir.dt.float32
    bf16 = mybir.dt.bfloat16
    RELU = mybir.ActivationFunctionType.Relu
    ADD = mybir.AluOpType.add
    BYPASS = mybir.AluOpType.bypass

    nc = bacc.Bacc(
        "TRN2",
        target_bir_lowering=False,
        debug=False,
        enable_asserts=False,
        num_devices=M,
    )

    # ---- I/O -------------------------------------------------------------
    AclT = nc.dram_tensor("AclT", [N, R], bf16, kind="ExternalInput")
    AueT = nc.dram_tensor("AueT", [N, R], bf16, kind="ExternalInput")
    X1T_d = nc.dram_tensor("X1T", [3, N], f32, kind="ExternalInput")
    X2T_d = nc.dram_tensor("X2T", [3, N], f32, kind="ExternalInput")
    XueT_d = nc.dram_tensor("XueT", [3, N], f32, kind="ExternalInput")
    w10_d = nc.dram_tensor("w10", [3, D], f32, kind="ExternalInput")
    w20_d = nc.dram_tensor("w20", [3, D], f32, kind="ExternalInput")
    w30_d = nc.dram_tensor("w30", [3, D], f32, kind="ExternalInput")
    w1x_d = nc.dram_tensor("w1x", [D + 1, 3, D], f32, kind="ExternalInput")
    w2x_d = nc.dram_tensor("w2x", [D + 1, 3, D], f32, kind="ExternalInput")
    w3x_d = nc.dram_tensor("w3x", [D + 1, 3, D], f32, kind="ExternalInput")
    q1x_d = nc.dram_tensor("q1x", [D + 1, D], f32, kind="ExternalInput")
    q2x_d = nc.dram_tensor("q2x", [D + 1, 1], f32, kind="ExternalInput")
    out_d = nc.dram_tensor("out", [1, 1], f32, kind="ExternalOutput")

    # internal DRAM for collectives
    Lg = nc.dram_tensor("Lg", [JT, P, 2 * D], bf16)
    Gg = nc.dram_tensor("Gg", [KT, P, 2 * D], bf16, addr_space="Shared")
    prd_l = nc.dram_tensor("prd_l", [D, 1], f32)
    prd_s = nc.dram_tensor("prd_s", [D, 1], f32, addr_space="Shared")

    groups = [list(range(M))]
    reps = int(os.environ.get("KREPS", "1"))
    nocc = bool(int(os.environ.get("KNOCC", "0")))  # no collectives (timing)
    kmode = os.environ.get("KMODE", "full")         # full | dmaonly

    with tile.TileContext(nc) as tc, tc.tile_pool(name="persist", bufs=1) as pp:
        # persistent SBUF state
        Hbuf = pp.tile([P, KT, 2 * D], bf16, tag="Hbuf")  # [:,k,0:64]=H_ue, 64:128=H_cl
        w10 = pp.tile([3, D], f32, tag="w10s")
        w20 = pp.tile([3, D], f32, tag="w20s")
        w30 = pp.tile([3, D], f32, tag="w30s")
        w1x = pp.tile([D + 1, 3, D], f32, tag="w1xs")
        w2x = pp.tile([D + 1, 3, D], f32, tag="w2xs")
        w3x = pp.tile([D + 1, 3, D], f32, tag="w3xs")
        q1x = pp.tile([D + 1, D], f32, tag="q1xs")
        q2x = pp.tile([D + 1, 1], f32, tag="q2xs")
        ones_mv = pp.tile([P, 1], f32, tag="ones_mv")

        nc.sync.dma_start(out=w10[:], in_=w10_d[:])
        nc.sync.dma_start(out=w20[:], in_=w20_d[:])
        nc.sync.dma_start(out=w30[:], in_=w30_d[:])
        nc.sync.dma_start(out=w1x[:], in_=w1x_d[:])
        nc.sync.dma_start(out=w2x[:], in_=w2x_d[:])
        nc.sync.dma_start(out=w3x[:], in_=w3x_d[:])
        nc.sync.dma_start(out=q1x[:], in_=q1x_d[:])
        nc.sync.dma_start(out=q2x[:], in_=q2x_d[:])
        nc.gpsimd.memset(ones_mv[:], 1.0)

        with (
            tc.tile_pool(name="pa", bufs=5) as pa,
            tc.tile_pool(name="pb", bufs=5) as pb,
            tc.tile_pool(name="ps", bufs=1, space="PSUM") as ps,
            tc.tile_pool(name="sbE", bufs=2) as sbE,
            tc.tile_pool(name="pX", bufs=2) as pX,
        ):
          if kmode != "full":
              nc.gpsimd.memset(Hbuf[:], 0.0)
          for _rep in range(reps):
            # ---- layer 0: full H0 for all N rows, interleaved into Hbuf --
            for g in range(8 if kmode == "full" else 0):
                gsl = slice(g * R, (g + 1) * R)
                x1c = pX.tile([3, R], f32, tag="x1c")
                x2c = pX.tile([3, R], f32, tag="x2c")
                xuc = pX.tile([3, R], f32, tag="xuc")
                nc.sync.dma_start(out=x1c[:], in_=X1T_d[:, gsl])
                nc.sync.dma_start(out=x2c[:], in_=X2T_d[:, gsl])
                nc.sync.dma_start(out=xuc[:], in_=XueT_d[:, gsl])
                pue = ps.tile([P, 8, D], f32, tag="pnue")
                pc1 = ps.tile([P, 8, D], f32, tag="pn1")
                pc2 = ps.tile([P, 8, D], f32, tag="pn2")
                for jj in range(8):
                    sl = slice(jj * P, (jj + 1) * P)
                    nc.tensor.matmul(pue[:, jj, :], xuc[:, sl], w30[:], start=True, stop=True)
                    nc.tensor.matmul(pc1[:, jj, :], x1c[:, sl], w10[:], start=True, stop=True)
                    nc.tensor.matmul(pc2[:, jj, :], x2c[:, sl], w20[:], start=True, stop=True)
                jsl = slice(g * 8, (g + 1) * 8)
                t1 = sbE.tile([P, 8, D], f32, tag="t1")
                t2 = sbE.tile([P, 8, D], f32, tag="t2")
                nc.scalar.activation(Hbuf[:, jsl, 0:D], pue[:], RELU)
                nc.scalar.activation(t1[:], pc1[:], RELU)
                nc.scalar.activation(t2[:], pc2[:], RELU)
                nc.vector.tensor_tensor(Hbuf[:, jsl, D : 2 * D], t1[:], t2[:], ADD)

            # ---- main layers ---------------------------------------------
            for l in range(3):
                last = l == 2
                mue = P if not last else D  # ue-pass stationary width
                Pcl0 = ps.tile([D, 512], f32, tag="acc_cl0")
                Pcl1 = ps.tile([D, 512], f32, tag="acc_cl1")
                Pue0 = ps.tile([mue, 512], f32, tag="acc_ue0")
                Pue1 = ps.tile([mue, 512], f32, tag="acc_ue1")
                for kb in range(KT // KB):
                    rows = slice(kb * KB * P, (kb + 1) * KB * P)
                    at = pa.tile([P, KB, R], bf16, tag="acl")
                    bt = pb.tile([P, KB, R], bf16, tag="aue")
                    nc.sync.dma_start(
                        out=at[:], in_=AclT[rows, :].rearrange("(kk p) r -> p kk r", p=P)
                    )
                    nc.sync.dma_start(
                        out=bt[:], in_=AueT[rows, :].rearrange("(kk p) r -> p kk r", p=P)
                    )
                    for kk in range(KB):
                        k = kb * KB + kk
                        st_cl = Hbuf[:, k, D : 2 * D]
                        st_ue = Hbuf[:, k, 0:mue]
                        s, e = k == 0, k == KT - 1
                        nc.tensor.matmul(Pcl0[:], st_cl, at[:, kk, 0:512], start=s, stop=e)
                        nc.tensor.matmul(Pue0[:], st_ue, bt[:, kk, 0:512], start=s, stop=e)
                        if kmode == "full":
                            nc.tensor.matmul(Pcl1[:], st_cl, at[:, kk, 512:1024], start=s, stop=e)
                            nc.tensor.matmul(Pue1[:], st_ue, bt[:, kk, 512:1024], start=s, stop=e)
                if kmode != "full":
                    continue

                # epilogue: X^T blocks -> next-layer H for this core's rows
                XT1 = sbE.tile([D + 1, R], f32, tag="xt1")
                XT2 = sbE.tile([D + 1, R], f32, tag="xt2")
                nc.vector.tensor_copy(XT1[0:D, 0:512], Pcl0[:])
                nc.vector.tensor_copy(XT1[0:D, 512:1024], Pcl1[:])
                nc.gpsimd.memset(XT1[D : D + 1, :], 1.0)
                nc.vector.tensor_copy(XT2[0:D, 0:512], Pue0[0:D, :])
                nc.vector.tensor_copy(XT2[0:D, 512:1024], Pue1[0:D, :])
                nc.gpsimd.memset(XT2[D : D + 1, :], 1.0)
                if not last:
                    XT3 = sbE.tile([D + 1, R], f32, tag="xt3")
                    nc.vector.tensor_copy(XT3[0:D, 0:512], Pue0[D:P, :])
                    nc.vector.tensor_copy(XT3[0:D, 512:1024], Pue1[D:P, :])
                    nc.gpsimd.memset(XT3[D : D + 1, :], 1.0)

                Pn1 = ps.tile([P, 8, D], f32, tag="pn1")
                Pn2 = ps.tile([P, 8, D], f32, tag="pn2")
                if not last:
                    Pnue = ps.tile([P, 8, D], f32, tag="pnue")
                for jj in range(JT):
                    sl = slice(jj * P, (jj + 1) * P)
                    nc.tensor.matmul(Pn1[:, jj, :], XT1[:, sl], w1x[:, l, :], start=True, stop=True)
                    nc.tensor.matmul(Pn2[:, jj, :], XT2[:, sl], w2x[:, l, :], start=True, stop=True)
                    if not last:
                        nc.tensor.matmul(Pnue[:, jj, :], XT3[:, sl], w3x[:, l, :], start=True, stop=True)

                t1 = sbE.tile([P, 8, D], f32, tag="t1")
                t2 = sbE.tile([P, 8, D], f32, tag="t2")
                nc.scalar.activation(t1[:], Pn1[:], RELU)
                nc.scalar.activation(t2[:], Pn2[:], RELU)

                if not last:
                    Epad = sbE.tile([P, JT, 2 * D], bf16, tag="epad")
                    nc.scalar.activation(Epad[:, :, 0:D], Pnue[:], RELU)
                    nc.vector.tensor_tensor(Epad[:, :, D : 2 * D], t1[:], t2[:], ADD)
                    for jj in range(JT):
                        nc.sync.dma_start(out=Lg[jj], in_=Epad[:, jj, :])
                    if nocc:
                        nc.sync.dma_start(out=Gg[0:JT], in_=Lg[:])
                    else:
                        nc.gpsimd.collective_compute(
                            "AllGather",
                            BYPASS,
                            replica_groups=groups,
                            ins=[Lg[:].opt()],
                            outs=[Gg[:].opt()],
                        )
                    nc.sync.dma_start(
                        out=Hbuf[:], in_=Gg[:].rearrange("j p c -> p j c")
                    )
                else:
                    # H_cl3 block -> column sum -> AllReduce -> head MLP
                    hs = sbE.tile([P, JT, D], f32, tag="hs")
                    nc.vector.tensor_tensor(hs[:], t1[:], t2[:], ADD)
                    Ppool = ps.tile([D, 1], f32, tag="pooled")
                    for jj in range(JT):
                        nc.tensor.matmul(
                            Ppool[:], hs[:, jj, :], ones_mv[:],
                            start=(jj == 0), stop=(jj == JT - 1),
                        )
                    pl_s = sbE.tile([D, 1], f32, tag="pl")
                    nc.vector.tensor_copy(pl_s[:], Ppool[:])
                    nc.sync.dma_start(out=prd_l[:], in_=pl_s[:])
                    if nocc:
                        nc.sync.dma_start(out=prd_s[:], in_=prd_l[:])
                    else:
                        nc.gpsimd.collective_compute(
                            "AllReduce",
                            ADD,
                            replica_groups=groups,
                            ins=[prd_l[:].opt()],
                            outs=[prd_s[:].opt()],
                        )
                    pvec = sbE.tile([D + 1, 1], f32, tag="pvec")
                    nc.sync.dma_start(out=pvec[0:D, :], in_=prd_s[:])
                    nc.gpsimd.memset(pvec[D : D + 1, :], 1.0)
                    Pz = ps.tile([D, 1], f32, tag="pooled")
                    nc.tensor.matmul(Pz[:], q1x[:], pvec[:], start=True, stop=True)
                    zt = sbE.tile([D + 1, 1], f32, tag="zt")
                    nc.scalar.activation(zt[0:D, :], Pz[:], RELU)
                    nc.gpsimd.memset(zt[D : D + 1, :], 1.0)
                    Po = ps.tile([1, 1], f32, tag="pooled")
                    nc.tensor.matmul(Po[:], q2x[:], zt[:], start=True, stop=True)
                    o_s = sbE.tile([1, 1], f32, tag="os")
                    nc.vector.tensor_copy(o_s[:], Po[:])
                    nc.sync.dma_start(out=out_d[:], in_=o_s[:])

            if kmode != "full":
                o_s = sbE.tile([1, 1], f32, tag="os")
                nc.gpsimd.memset(o_s[:], 0.0)
                nc.sync.dma_start(out=out_d[:], in_=o_s[:])

    nc.compile()
    return nc


def _get_module():
    global _CACHED
    if _CACHED is None:
        _CACHED = _build_module()
    return _CACHED


def prep_in_maps(inputs):
    import ml_dtypes

    f = np.float32
    bf = ml_dtypes.bfloat16
    A_cl = np.asarray(inputs["A_cl"], f)
    A_ue = np.asarray(inputs["A_ue"], f)
    ones_row = np.ones((1, N), f)
    X1T = np.ascontiguousarray(np.vstack([np.asarray(inputs["X_cl_1"], f).T, ones_row]))
    X2T = np.ascontiguousarray(np.vstack([np.asarray(inputs["X_cl_2"], f).T, ones_row]))
    XueT = np.ascontiguousarray(np.vstack([np.asarray(inputs["X_ue"], f).T, ones_row]))

    def wx0(w, b):
        return np.ascontiguousarray(np.vstack([np.asarray(w, f), np.asarray(b, f)[None, :]]))

    def wx(w, b):
        # [3, D, D] + [3, D] -> [D+1, 3, D]
        w = np.asarray(w, f)
        b = np.asarray(b, f)
        stk = np.stack([np.vstack([w[i], b[i][None, :]]) for i in range(3)], axis=1)
        return np.ascontiguousarray(stk)

    common = {
        "X1T": X1T,
        "X2T": X2T,
        "XueT": XueT,
        "w10": wx0(inputs["W1_w0"], inputs["W1_b0"]),
        "w20": wx0(inputs["W2_w0"], inputs["W2_b0"]),
        "w30": wx0(inputs["W3_w0"], inputs["W3_b0"]),
        "w1x": wx(inputs["W1_w"], inputs["W1_b"]),
        "w2x": wx(inputs["W2_w"], inputs["W2_b"]),
        "w3x": wx(inputs["W3_w"], inputs["W3_b"]),
        "q1x": wx0(inputs["Q_w1"], inputs["Q_b1"]),
        "q2x": np.ascontiguousarray(
            np.vstack([np.asarray(inputs["Q_w2"], f), np.asarray(inputs["Q_b2"], f)[None, :]])
        ),
    }

    in_maps = []
    for c in range(M):
        rs = slice(c * R, (c + 1) * R)
        m = dict(common)
        m["AclT"] = np.ascontiguousarray(A_cl[rs, :].T.astype(bf))
        m["AueT"] = np.ascontiguousarray(A_ue[rs, :].T.astype(bf))
        in_maps.append(m)
    return in_maps


def kernel(**inputs):
    global LAST_EXEC_NS, LAST_PROFILE
    nc = _get_module()
    from concourse.bass_utils import run_bass_kernel_spmd

    trace = os.environ.get("BASS_KERNEL_TRACE", "0").lower() in ("1", "true", "yes")
    tmpdir = os.environ.get("KTMPDIR") or None
    in_maps = prep_in_maps(inputs)
    res = run_bass_kernel_spmd(
        nc, in_maps, core_ids=list(range(M)), trace=trace, tmpdir=tmpdir
    )
    LAST_EXEC_NS = res.exec_time_ns
    LAST_PROFILE = res.profile_json
    return np.asarray(res.results[0]["out"], np.float32)



# revision 3
# speedup vs baseline: 2.0006x; 2.0006x over previous
"""Trainium2 Bass kernel for nn_GNN_37615323579234 (gnn_message_passing), v2.

Math (reference, N=8192, D=64, 4 layers; layer-3 A@H products are dead code):
    l=0..3:  H_cl = relu(X1@w1+b1) + relu(X2@w2+b2);  H_ue = relu(Xue@w3+b3)
             X1 = A_cl@H_cl;  X2 = A_ue@H_ue;  Xue = A_ue@H_cl
    out = relu(colsum(H_cl3) @ Qw1 + Qb1) @ Qw2 + Qb2      # [1,1]

v2 design (vs bf16-streaming baseline):
  * A row-blocks are pre-scaled by N (values land in [0,1)) and cast to
    fp8 e4m3 on host, laid out [P, KT, R] = [part, k-tile, row] so each
    core's A^T block is 8 MiB -> BOTH matrices live in SBUF persistently
    (128 KiB/partition).  They are streamed from HBM exactly once,
    overlapped with layer-1 matmuls; layers 2-3 do zero A-matrix DMA.
  * H is stored fp8 (interleaved [H_ue | H_cl] per k-tile) so the big
    matmuls run in DoubleRow fp8 perf mode: each instruction contracts
    2 k-tiles (256 rows) at 2x bf16 throughput.  The 1/N de-scale is
    folded into the layer>=1 weight matrices host-side.
  * Layer-0 (H0 from the [N,2] inputs) runs in bf16 and is interleaved
    group-by-group with layer-1's big matmuls so it hides under the A
    stream-in.
  * Between layers: AllGather of the 128 KiB fp8 H block (single-DMA
    staging, [M,P,JT,2D] gather layout so the SBUF reload uses 1 KiB
    descriptors), AllReduce of the pooled [D] vector at the end.
"""

import os
import sys

for _p in ("/opt/trn_rl_repo", "/root/.axon_site/_ro/trn_rl_repo"):
    if os.path.isdir(_p) and _p not in sys.path:
        sys.path.insert(0, _p)

import numpy as np

N = 8192
D = 64
M = 8          # cores
R = N // M     # 1024 rows per core
P = 128        # partitions
KT = N // P    # 64 k-tiles
JT = R // P    # 8 row-tiles per core
KB = 4         # k-tiles per A-stream DMA chunk (512 KiB fp8)
NCH = KT // KB  # 16 chunks per matrix

LAST_EXEC_NS = None
LAST_PROFILE = None

_CACHED = None  # compile once per process


def _build_module():
    import concourse.bacc as bacc
    import concourse.mybir as mybir
    from concourse import tile

    f32 = mybir.dt.float32
    bf16 = mybir.dt.bfloat16
    f8 = mybir.dt.float8e4
    RELU = mybir.ActivationFunctionType.Relu
    ADD = mybir.AluOpType.add
    BYPASS = mybir.AluOpType.bypass
    DR = mybir.MatmulPerfMode.DoubleRow

    nc = bacc.Bacc(
        "TRN2",
        target_bir_lowering=False,
        debug=False,
        enable_asserts=False,
        num_devices=M,
    )

    # ---- I/O -------------------------------------------------------------
    Acl8_d = nc.dram_tensor("Acl8", [P, KT, R], f8, kind="ExternalInput")
    Aue8_d = nc.dram_tensor("Aue8", [P, KT, R], f8, kind="ExternalInput")
    XTall_d = nc.dram_tensor("XTall", [3, 3, N], bf16, kind="ExternalInput")
    w10_d = nc.dram_tensor("w10", [3, D], bf16, kind="ExternalInput")
    w20_d = nc.dram_tensor("w20", [3, D], bf16, kind="ExternalInput")
    w30_d = nc.dram_tensor("w30", [3, D], bf16, kind="ExternalInput")
    w1x_d = nc.dram_tensor("w1x", [D + 1, 3, D], bf16, kind="ExternalInput")
    w2x_d = nc.dram_tensor("w2x", [D + 1, 3, D], bf16, kind="ExternalInput")
    w3x_d = nc.dram_tensor("w3x", [D + 1, 3, D], bf16, kind="ExternalInput")
    q1x_d = nc.dram_tensor("q1x", [D + 1, D], f32, kind="ExternalInput")
    q2x_d = nc.dram_tensor("q2x", [D + 1, 1], f32, kind="ExternalInput")
    out_d = nc.dram_tensor("out", [1, 1], f32, kind="ExternalOutput")

    # internal DRAM for collectives
    Lg = nc.dram_tensor("Lg", [P, JT, 2 * D], f8)
    Gg = nc.dram_tensor("Gg", [M, P, JT, 2 * D], f8, addr_space="Shared")
    prd_l = nc.dram_tensor("prd_l", [D, 1], f32)
    prd_s = nc.dram_tensor("prd_s", [D, 1], f32, addr_space="Shared")

    groups = [list(range(M))]
    reps = int(os.environ.get("KREPS", "1"))
    nocc = bool(int(os.environ.get("KNOCC", "0")))  # no collectives (timing)

    with tile.TileContext(nc) as tc, tc.tile_pool(name="persist", bufs=1) as pp:
        # persistent SBUF state
        Acl_sb = pp.tile([P, KT, R], f8, tag="Acl_sb")
        Aue_sb = pp.tile([P, KT, R], f8, tag="Aue_sb")
        Hbuf = pp.tile([P, KT, 2 * D], f8, tag="Hbuf")  # [:,k,0:64]=H_ue, 64:128=H_cl
        xh0 = pp.tile([3, 3, N // 2], bf16, tag="xh0")
        xh1 = pp.tile([3, 3, N // 2], bf16, tag="xh1")
        xh = (xh0, xh1)
        w10 = pp.tile([3, D], bf16, tag="w10s")
        w20 = pp.tile([3, D], bf16, tag="w20s")
        w30 = pp.tile([3, D], bf16, tag="w30s")
        w1x = pp.tile([D + 1, 3, D], bf16, tag="w1xs")
        w2x = pp.tile([D + 1, 3, D], bf16, tag="w2xs")
        w3x = pp.tile([D + 1, 3, D], bf16, tag="w3xs")
        q1x = pp.tile([D + 1, D], f32, tag="q1xs")
        q2x = pp.tile([D + 1, 1], f32, tag="q2xs")
        ones_mv = pp.tile([P, 1], f32, tag="ones_mv")

        # small preloads on the scalar (ACT) HWDGE ring so they are not
        # queued behind the 16 MiB A stream on the sync ring
        nc.scalar.dma_start(out=w10[:], in_=w10_d[:])
        nc.scalar.dma_start(out=w20[:], in_=w20_d[:])
        nc.scalar.dma_start(out=w30[:], in_=w30_d[:])
        nc.scalar.dma_start(out=xh0[:], in_=XTall_d[:, :, 0 : N // 2])
        nc.scalar.dma_start(out=xh1[:], in_=XTall_d[:, :, N // 2 : N])
        nc.scalar.dma_start(out=w1x[:], in_=w1x_d[:])
        nc.scalar.dma_start(out=w2x[:], in_=w2x_d[:])
        nc.scalar.dma_start(out=w3x[:], in_=w3x_d[:])
        nc.scalar.dma_start(out=q1x[:], in_=q1x_d[:])
        nc.scalar.dma_start(out=q2x[:], in_=q2x_d[:])
        nc.gpsimd.memset(ones_mv[:], 1.0)

        with (
            tc.tile_pool(name="ps", bufs=1, space="PSUM") as ps,
            tc.tile_pool(name="sbE", bufs=1) as sbE,
        ):
          for rep in range(reps):
            # ---- A stream-in: both matrices, interleaved chunks --------
            if rep == 0:
                for c in range(NCH):
                    ksl = slice(c * KB, (c + 1) * KB)
                    nc.sync.dma_start(out=Acl_sb[:, ksl, :], in_=Acl8_d[:, ksl, :])
                    nc.sync.dma_start(out=Aue_sb[:, ksl, :], in_=Aue8_d[:, ksl, :])

            def emit_l0_group(g):
                half = xh[g // 4]
                base = (g % 4) * R
                pue = ps.tile([P, JT, D], f32, tag="pnue")
                pc1 = ps.tile([P, JT, D], f32, tag="pn1")
                pc2 = ps.tile([P, JT, D], f32, tag="pn2")
                for jj in range(JT):
                    sl = slice(base + jj * P, base + (jj + 1) * P)
                    nc.tensor.matmul(pue[:, jj, :], half[:, 2, sl], w30[:], start=True, stop=True)
                    nc.tensor.matmul(pc1[:, jj, :], half[:, 0, sl], w10[:], start=True, stop=True)
                    nc.tensor.matmul(pc2[:, jj, :], half[:, 1, sl], w20[:], start=True, stop=True)
                jsl = slice(g * JT, (g + 1) * JT)
                t1 = sbE.tile([P, JT, D], f32, tag="t1")
                t2 = sbE.tile([P, JT, D], f32, tag="t2")
                nc.scalar.activation(Hbuf[:, jsl, 0:D], pue[:], RELU)
                nc.scalar.activation(t1[:], pc1[:], RELU)
                nc.scalar.activation(t2[:], pc2[:], RELU)
                nc.vector.tensor_tensor(Hbuf[:, jsl, D : 2 * D], t1[:], t2[:], ADD)

            # ---- main layers ---------------------------------------------
            for l in range(3):
                last = l == 2
                wue = 2 * D if not last else D  # ue-pass stationary width
                Pcl0 = ps.tile([D, 512], f32, tag="acc_cl0")
                Pcl1 = ps.tile([D, 512], f32, tag="acc_cl1")
                Pue0 = ps.tile([wue, 512], f32, tag="acc_ue0")
                Pue1 = ps.tile([wue, 512], f32, tag="acc_ue1")
                for k2 in range(KT // 2):
                    if l == 0 and k2 % 4 == 0:
                        emit_l0_group(k2 // 4)
                    s, e = k2 == 0, k2 == KT // 2 - 1
                    k0 = 2 * k2
                    st_cl = Hbuf[:, k0 : k0 + 2, D : 2 * D]
                    st_ue = Hbuf[:, k0 : k0 + 2, 0:wue]
                    acl = Acl_sb[:, k0 : k0 + 2, :]
                    aue = Aue_sb[:, k0 : k0 + 2, :]
                    nc.tensor.matmul(Pcl0[:], st_cl, acl[:, :, 0:512], start=s, stop=e, perf_mode=DR)
                    nc.tensor.matmul(Pcl1[:], st_cl, acl[:, :, 512:1024], start=s, stop=e, perf_mode=DR)
                    nc.tensor.matmul(Pue0[:], st_ue, aue[:, :, 0:512], start=s, stop=e, perf_mode=DR)
                    nc.tensor.matmul(Pue1[:], st_ue, aue[:, :, 512:1024], start=s, stop=e, perf_mode=DR)

                # epilogue: X^T blocks -> next-layer H for this core's rows
                XT1 = sbE.tile([D + 1, R], bf16, tag="xt1")
                XT2 = sbE.tile([D + 1, R], bf16, tag="xt2")
                nc.vector.tensor_copy(XT1[0:D, 0:512], Pcl0[:])
                nc.vector.tensor_copy(XT1[0:D, 512:1024], Pcl1[:])
                nc.gpsimd.memset(XT1[D : D + 1, :], 1.0)
                nc.vector.tensor_copy(XT2[0:D, 0:512], Pue0[0:D, :])
                nc.vector.tensor_copy(XT2[0:D, 512:1024], Pue1[0:D, :])
                nc.gpsimd.memset(XT2[D : D + 1, :], 1.0)
                if not last:
                    XT3 = sbE.tile([D + 1, R], bf16, tag="xt3")
                    nc.vector.tensor_copy(XT3[0:D, 0:512], Pue0[D:P, :])
                    nc.vector.tensor_copy(XT3[0:D, 512:1024], Pue1[D:P, :])
                    nc.gpsimd.memset(XT3[D : D + 1, :], 1.0)

                Pn1 = ps.tile([P, JT, D], f32, tag="pn1")
                Pn2 = ps.tile([P, JT, D], f32, tag="pn2")
                if not last:
                    Pnue = ps.tile([P, JT, D], f32, tag="pnue")
                for jj in range(JT):
                    sl = slice(jj * P, (jj + 1) * P)
                    nc.tensor.matmul(Pn1[:, jj, :], XT1[:, sl], w1x[:, l, :], start=True, stop=True)
                    nc.tensor.matmul(Pn2[:, jj, :], XT2[:, sl], w2x[:, l, :], start=True, stop=True)
                    if not last:
                        nc.tensor.matmul(Pnue[:, jj, :], XT3[:, sl], w3x[:, l, :], start=True, stop=True)

                t1 = sbE.tile([P, JT, D], f32, tag="t1")
                t2 = sbE.tile([P, JT, D], f32, tag="t2")
                nc.scalar.activation(t1[:], Pn1[:], RELU)
                nc.scalar.activation(t2[:], Pn2[:], RELU)

                if not last:
                    Epad = sbE.tile([P, JT, 2 * D], f8, tag="epad")
                    nc.scalar.activation(Epad[:, :, 0:D], Pnue[:], RELU)
                    nc.vector.tensor_tensor(Epad[:, :, D : 2 * D], t1[:], t2[:], ADD)
                    nc.sync.dma_start(out=Lg[:], in_=Epad[:])
                    if nocc:
                        nc.sync.dma_start(out=Gg[0], in_=Lg[:])
                    else:
                        nc.gpsimd.collective_compute(
                            "AllGather",
                            BYPASS,
                            replica_groups=groups,
                            ins=[Lg[:].opt()],
                            outs=[Gg[:].opt()],
                        )
                    for c in range(M):
                        nc.sync.dma_start(
                            out=Hbuf[:, c * JT : (c + 1) * JT, :], in_=Gg[c]
                        )
                else:
                    # H_cl3 block -> column sum -> AllReduce -> head MLP
                    hs = sbE.tile([P, JT, D], f32, tag="hs")
                    nc.vector.tensor_tensor(hs[:], t1[:], t2[:], ADD)
                    Ppool = ps.tile([D, 1], f32, tag="pooled")
                    for jj in range(JT):
                        nc.tensor.matmul(
                            Ppool[:], hs[:, jj, :], ones_mv[:],
                            start=(jj == 0), stop=(jj == JT - 1),
                        )
                    pl_s = sbE.tile([D, 1], f32, tag="pl")
                    nc.vector.tensor_copy(pl_s[:], Ppool[:])
                    nc.sync.dma_start(out=prd_l[:], in_=pl_s[:])
                    if nocc:
                        nc.sync.dma_start(out=prd_s[:], in_=prd_l[:])
                    else:
                        nc.gpsimd.collective_compute(
                            "AllReduce",
                            ADD,
                            replica_groups=groups,
                            ins=[prd_l[:].opt()],
                            outs=[prd_s[:].opt()],
                        )
                    pvec = sbE.tile([D + 1, 1], f32, tag="pvec")
                    nc.sync.dma_start(out=pvec[0:D, :], in_=prd_s[:])
                    nc.gpsimd.memset(pvec[D : D + 1, :], 1.0)
                    Pz = ps.tile([D, 1], f32, tag="pooled")
                    nc.tensor.matmul(Pz[:], q1x[:], pvec[:], start=True, stop=True)
                    zt = sbE.tile([D + 1, 1], f32, tag="zt")
                    nc.scalar.activation(zt[0:D, :], Pz[:], RELU)
                    nc.gpsimd.memset(zt[D : D + 1, :], 1.0)
                    Po = ps.tile([1, 1], f32, tag="pooled")
                    nc.tensor.matmul(Po[:], q2x[:], zt[:], start=True, stop=True)
                    o_s = sbE.tile([1, 1], f32, tag="os")
                    nc.vector.tensor_copy(o_s[:], Po[:])
                    nc.sync.dma_start(out=out_d[:], in_=o_s[:])

    nc.compile()
    return nc


def _get_module():
    global _CACHED
    if _CACHED is None:
        _CACHED = _build_module()
    return _CACHED


def prep_in_maps(inputs):
    import ml_dtypes

    f = np.float32
    bf = ml_dtypes.bfloat16
    f8 = ml_dtypes.float8_e4m3

    A_cl = np.asarray(inputs["A_cl"], f)
    A_ue = np.asarray(inputs["A_ue"], f)

    # X features + ones row, [row(2+1), tensor(3), N] in bf16
    XTall = np.empty((3, 3, N), f)
    for t, name in enumerate(("X_cl_1", "X_cl_2", "X_ue")):
        x = np.asarray(inputs[name], f)
        XTall[0, t, :] = x[:, 0]
        XTall[1, t, :] = x[:, 1]
    XTall[2, :, :] = 1.0
    XTall = XTall.astype(bf)

    def wx0(w, b):
        return np.vstack([np.asarray(w, f), np.asarray(b, f)[None, :]]).astype(bf)

    def wxs(w, b):
        # [3, D, D] + [3, D] -> [D+1, 3, D]; weight rows pre-divided by N to
        # undo the fp8 A pre-scale (bias row untouched)
        w = np.asarray(w, f) * np.float32(1.0 / N)
        b = np.asarray(b, f)
        stk = np.stack([np.vstack([w[i], b[i][None, :]]) for i in range(3)], axis=1)
        return np.ascontiguousarray(stk.astype(bf))

    common = {
        "XTall": XTall,
        "w10": wx0(inputs["W1_w0"], inputs["W1_b0"]),
        "w20": wx0(inputs["W2_w0"], inputs["W2_b0"]),
        "w30": wx0(inputs["W3_w0"], inputs["W3_b0"]),
        "w1x": wxs(inputs["W1_w"], inputs["W1_b"]),
        "w2x": wxs(inputs["W2_w"], inputs["W2_b"]),
        "w3x": wxs(inputs["W3_w"], inputs["W3_b"]),
        "q1x": np.ascontiguousarray(
            np.vstack([np.asarray(inputs["Q_w1"], f), np.asarray(inputs["Q_b1"], f)[None, :]])
        ),
        "q2x": np.ascontiguousarray(
            np.vstack([np.asarray(inputs["Q_w2"], f), np.asarray(inputs["Q_b2"], f)[None, :]])
        ),
    }

    def ablk(A, c):
        # core c's row block, transposed, pre-scaled by N, [P, KT, R] fp8:
        # out[p, k, r] = N * A[c*R + r, k*128 + p]
        t = A[c * R : (c + 1) * R, :].T * np.float32(N)   # [N, R]
        t = t.reshape(KT, P, R).transpose(1, 0, 2)         # [P, KT, R]
        return np.ascontiguousarray(t.astype(f8))

    in_maps = []
    for c in range(M):
        m = dict(common)
        m["Acl8"] = ablk(A_cl, c)
        m["Aue8"] = ablk(A_ue, c)
        in_maps.append(m)
    return in_maps


def kernel(**inputs):
    global LAST_EXEC_NS, LAST_PROFILE
    nc = _get_module()
    from concourse.bass_utils import run_bass_kernel_spmd

    trace = os.environ.get("BASS_KERNEL_TRACE", "0").lower() in ("1", "true", "yes")
    tmpdir = os.environ.get("KTMPDIR") or None
    in_maps = prep_in_maps(inputs)
    res = run_bass_kernel_spmd(
        nc, in_maps, core_ids=list(range(M)), trace=trace, tmpdir=tmpdir
    )
    LAST_EXEC_NS = res.exec_time_ns
    LAST_PROFILE = res.profile_json
    return np.asarray(res.results[0]["out"], np.float32)
